# revision 9
# baseline (speedup 1.0000x reference)
"""Bass/Trainium2 kernel for 2-layer GAT (nn_GAT_18640158065247).

Strategy (8 NeuronCores, SPMD single program):
 - Host: add self-loops, sort edges by dst, shard dst-contiguously so each core
   owns 49 aligned 128-node chunks (50176 padded nodes) and the edges into them.
 - Phase A (replicated on every core): T[n] = [H bf16 | alpha_src f32] table,
   AD[n] = alpha_dst bf16 table, via X^T PE-transpose + fused matmul with
   W1_ext = [W1 | W1@a_src | W1@a_dst] and a rank-1 bias row.
 - Phase B (edge-parallel): per 128-node chunk, per 128-edge tile:
   indirect-gather T rows by src, build one-hot IND (dst-local vs iota),
   expand alpha_dst to edges via PE matmul with IND^T (DVE stream transpose),
   w = max(exp(e), exp(0.2 e)) [= exp(leaky_relu(e))], weight messages, and
   accumulate [Σ w·H | Σ w] into PSUM with one matmul per tile.
   Epilogue: normalize, ELU (as q = exp(min(x,0)) + max(x,0), with the -1
   folded into the layer-2 bias row), z2/alpha2 via a second fused matmul.
 - AllGather the tiny per-node layer-2 table T2 = [z2+b2 | a2src | a2dst].
 - Phase D: same edge loop for layer 2 on 16-byte rows, then log-softmax.
"""
import numpy as np

N = 50000
E = 800000
NFEAT = 128
NHID = 16
HEADS = 8
NCLASS = 2
NEG = 0.2
NCORES = 8
P = 128

_PROG_CACHE = {}


# ---------------------------------------------------------------- host helpers
def _host_prep(x, edge_index, W1, att_src1, att_dst1, bias1, W2, att_src2,
               att_dst2, bias2):
    n_chunks = -(-N // P)                      # 391 -> pad to multiple of NCORES
    n_chunks = -(-n_chunks // NCORES) * NCORES  # 392
    NP = n_chunks * P                           # 50176
    CPC = n_chunks // NCORES                    # 49
    NPC = CPC * P                               # 6272 nodes per core

    # the appended self-loops (one per node) are handled as an implicit,
    # gather-free tile per chunk; only the raw edges go through the sort.
    src = edge_index[0]
    dst = edge_index[1]
    order = np.argsort(dst, kind="stable")
    src_s = src[order].astype(np.int32)
    dst_s = dst[order].astype(np.int32)

    chunk_of_edge = dst_s // P                          # [Etot]
    counts = np.bincount(chunk_of_edge, minlength=n_chunks)
    starts = np.zeros(n_chunks + 1, np.int64)
    np.cumsum(counts, out=starts[1:])

    cnt_cj = counts.reshape(NCORES, CPC)
    S_slots = (-(-cnt_cj.max(axis=0) // P) * P).astype(np.int64)   # padded edges/slot
    tiles_per_slot = S_slots // P
    tile_base = np.zeros(CPC + 1, np.int64)
    np.cumsum(tiles_per_slot, out=tile_base[1:])
    Ttot = int(tile_base[-1])                     # tiles per core

    offs_src = np.zeros((NCORES, P, Ttot), np.int32)
    dstl = np.full((NCORES, P, Ttot), -1.0, np.float32)
    for c in range(NCORES):
        for j in range(CPC):
            g = c * CPC + j
            e0, cnt = starts[g], counts[g]
            Sp = int(S_slots[j])
            sl = np.zeros(Sp, np.int32)
            dl = np.full(Sp, -1.0, np.float32)
            sl[:cnt] = src_s[e0:e0 + cnt]
            dl[:cnt] = (dst_s[e0:e0 + cnt] - g * P).astype(np.float32)
            tb = int(tile_base[j])
            nt = Sp // P
            offs_src[c, :, tb:tb + nt] = sl.reshape(nt, P).T
            dstl[c, :, tb:tb + nt] = dl.reshape(nt, P).T

    # parameters (fused)
    va_src1 = (W1.reshape(NFEAT, HEADS, NHID) * att_src1[None]).sum(-1)  # [128,8]
    va_dst1 = (W1.reshape(NFEAT, HEADS, NHID) * att_dst1[None]).sum(-1)
    W1_ext = np.concatenate([W1, va_src1, va_dst1], axis=1).astype(np.float32)  # [128,144]
    b1row = np.concatenate([bias1, np.zeros(16, np.float32)])[None, :].astype(np.float32)

    va2s = W2 @ att_src2[0]            # [128]
    va2d = W2 @ att_dst2[0]
    W2_ext = np.concatenate([W2, va2s[:, None], va2d[:, None]], axis=1).astype(np.float32)  # [128,4]
    bias_ext2 = np.concatenate([bias2, np.zeros(2, np.float32)])
    b2row = (bias_ext2 - W2_ext.sum(0))[None, :].astype(np.float32)   # folds q-1 shift

    import ml_dtypes
    x_pad = np.zeros((NP, NFEAT), np.float32)
    x_pad[:N] = x
    x_loc = x_pad.reshape(NCORES, NPC, NFEAT).astype(ml_dtypes.bfloat16)
    W1_ext_bf = W1_ext.astype(ml_dtypes.bfloat16)
    b1row_bf = b1row.astype(ml_dtypes.bfloat16)
    iota_row = np.tile(np.arange(P, dtype=np.float32), (P, 1))

    meta = dict(NP=NP, CPC=CPC, NPC=NPC, Ttot=Ttot,
                tiles_per_slot=tuple(int(t) for t in tiles_per_slot),
                tile_base=tuple(int(t) for t in tile_base))
    arrays = dict(offs_src=offs_src, dstl=dstl,
                  W1_ext=W1_ext_bf, b1row=b1row_bf, W2_ext=W2_ext, b2row=b2row,
                  x_loc=x_loc, iota_row=iota_row)
    return meta, arrays


# ---------------------------------------------------------------- device build
def _build_program(meta):
    import sys
    if "/opt/trn_rl_repo" not in sys.path:
        sys.path.insert(0, "/opt/trn_rl_repo")
    import concourse.bass as bass
    import concourse.tile as tile
    from concourse import mybir
    from concourse.masks import make_identity

    f32 = mybir.dt.float32
    bf16 = mybir.dt.bfloat16
    i32 = mybir.dt.int32
    AF = mybir.ActivationFunctionType
    OP = mybir.AluOpType

    NP = meta["NP"]; CPC = meta["CPC"]; NPC = meta["NPC"]; Ttot = meta["Ttot"]
    tiles_per_slot = meta["tiles_per_slot"]; tile_base = meta["tile_base"]
    n_chunks = NP // P

    nc = bass.Bass(num_devices=NCORES)

    x_in = nc.dram_tensor("x_loc", [NPC, NFEAT], bf16, kind="ExternalInput")
    w1e_in = nc.dram_tensor("W1_ext", [NFEAT, 144], bf16, kind="ExternalInput")
    b1_in = nc.dram_tensor("b1row", [1, 144], bf16, kind="ExternalInput")
    w2e_in = nc.dram_tensor("W2_ext", [NFEAT, 4], f32, kind="ExternalInput")
    b2_in = nc.dram_tensor("b2row", [1, 4], f32, kind="ExternalInput")
    iota_in = nc.dram_tensor("iota_row", [P, P], f32, kind="ExternalInput")
    offs_in = nc.dram_tensor("offs_src", [P, Ttot], i32, kind="ExternalInput")
    dstl_in = nc.dram_tensor("dstl", [P, Ttot], f32, kind="ExternalInput")

    out_ext = nc.dram_tensor("out", [NPC, NCLASS], f32, kind="ExternalOutput")

    # internal DRAM
    T_loc = nc.dram_tensor("T_loc", [NPC, 144], bf16)
    T_tab = nc.dram_tensor("T_tab", [NP, 144], bf16, addr_space="Shared")
    AD_loc = nc.dram_tensor("AD_loc", [NPC, HEADS], bf16)
    T2_loc = nc.dram_tensor("T2_loc", [NPC, 4], f32)
    T2_full = nc.dram_tensor("T2_full", [NP, 4], f32, addr_space="Shared")

    with tile.TileContext(nc) as tc:
        with (
            tc.tile_pool(name="const", bufs=1) as cpool,
            tc.tile_pool(name="work", bufs=3) as wpool,
            tc.tile_pool(name="gath", bufs=2) as gpool,
            tc.tile_pool(name="ps", bufs=2, space="PSUM") as ps,
        ):
            # ---- constants
            w1e_sb = cpool.tile([NFEAT, 144], bf16)
            nc.sync.dma_start(out=w1e_sb[:], in_=w1e_in[:, :])
            b1_sb = cpool.tile([1, 144], bf16)
            nc.sync.dma_start(out=b1_sb[:], in_=b1_in[:, :])
            w2e_sb = cpool.tile([NFEAT, 4], f32)
            nc.sync.dma_start(out=w2e_sb[:], in_=w2e_in[:, :])
            b2_sb = cpool.tile([1, 4], f32)
            nc.sync.dma_start(out=b2_sb[:], in_=b2_in[:, :])
            iota_sb = cpool.tile([P, P], f32)
            nc.sync.dma_start(out=iota_sb[:], in_=iota_in[:, :])
            offs_sb = cpool.tile([P, Ttot], i32)
            nc.sync.dma_start(out=offs_sb[:], in_=offs_in[:, :])
            dstl_sb = cpool.tile([P, Ttot], f32)
            nc.sync.dma_start(out=dstl_sb[:], in_=dstl_in[:, :])
            ident = cpool.tile([P, P], f32)
            make_identity(nc, ident[:])
            ident_bf = cpool.tile([P, P], bf16)
            make_identity(nc, ident_bf[:])
            ones1 = cpool.tile([1, P], f32)
            nc.vector.memset(ones1[:], 1.0)
            ones1_bf = cpool.tile([1, P], bf16)
            nc.vector.memset(ones1_bf[:], 1.0)

            # ---- phase A: build T / AD tables (own shard only, then AllGather)
            for j in range(CPC):
                nb = j * P
                xt_sb = wpool.tile([P, P], bf16, tag="xtA")
                nc.sync.dma_start(out=xt_sb[:], in_=x_in[nb:nb + P, :],
                                  transpose=True)
                t_ps = ps.tile([P, 144], f32, space="PSUM", tag="t144")
                nc.tensor.matmul(t_ps[:], lhsT=xt_sb[:], rhs=w1e_sb[:],
                                 start=True, stop=False)
                nc.tensor.matmul(t_ps[:], lhsT=ones1_bf[:], rhs=b1_sb[:],
                                 start=False, stop=True)
                t_sb = wpool.tile([P, 144], bf16, tag="tsbA")
                nc.vector.tensor_copy(out=t_sb[:, 0:128], in_=t_ps[:, 0:128])
                nc.vector.tensor_copy(
                    out=t_sb[:, 128:144].bitcast(f32), in_=t_ps[:, 128:136])
                ad_sb = wpool.tile([P, HEADS], bf16, tag="adA")
                nc.scalar.activation(out=ad_sb[:], in_=t_ps[:, 136:144], func=AF.Copy)
                nc.sync.dma_start(out=T_loc[nb:nb + P, :], in_=t_sb[:])
                nc.sync.dma_start(out=AD_loc[nb:nb + P, :], in_=ad_sb[:])

            nc.gpsimd.collective_compute(
                "AllGather", mybir.AluOpType.bypass,
                ins=[T_loc[:, :]], outs=[T_tab[:, :]],
                replica_groups=[list(range(NCORES))])

            # ---- phase B: layer-1 edge aggregation over own 49 chunks
            for j in range(CPC):
                S = tiles_per_slot[j] + 1     # +1: implicit self-loop tile
                tb = tile_base[j]
                adc = wpool.tile([P, HEADS], bf16, tag="adcB")
                nc.sync.dma_start(out=adc[:], in_=AD_loc[j * P:(j + 1) * P, :])

                gbuf = gpool.tile([P, S * 144], bf16, tag="gB")
                g3 = gbuf[:].rearrange("p (s e) -> p s e", s=S)
                ind = gpool.tile([P, (S - 1) * P], bf16, tag="indB")
                ind3 = ind[:].rearrange("p (s n) -> p s n", s=S - 1)
                adst_ps = ps.tile([P, S * HEADS], f32, space="PSUM", tag="tadst")

                # tile 0: this chunk's self-loops — data is local, one-hot is I
                nc.sync.dma_start(out=g3[:, 0, :], in_=T_loc[j * P:(j + 1) * P, :])
                nc.vector.tensor_copy(out=adst_ps[:, 0:HEADS], in_=adc[:])
                for t in range(1, S):
                    nc.gpsimd.indirect_dma_start(
                        out=g3[:, t, :], out_offset=None, in_=T_tab[:, :],
                        in_offset=bass.IndirectOffsetOnAxis(
                            ap=offs_sb[:, tb + t - 1:tb + t], axis=0))
                    nc.vector.tensor_scalar(
                        out=ind3[:, t - 1, :], in0=iota_sb[:],
                        scalar1=dstl_sb[:, tb + t - 1:tb + t], scalar2=None,
                        op0=OP.is_equal)
                    indt_ps = ps.tile([P, P], bf16, space="PSUM", tag="t128")
                    nc.tensor.transpose(out=indt_ps[:], in_=ind3[:, t - 1, :],
                                        identity=ident_bf[:])
                    indt = wpool.tile([P, P], bf16, tag="indtB")
                    nc.vector.tensor_copy(out=indt[:], in_=indt_ps[:])
                    nc.tensor.matmul(adst_ps[:, t * HEADS:(t + 1) * HEADS],
                                     lhsT=indt[:], rhs=adc[:],
                                     start=True, stop=True)

                # e = asrc + adst (all tiles at once), w = max(exp(e), exp(.2e))
                asrc_v = g3[:, :, 128:144].bitcast(f32)       # [P, S, 8]
                ep = wpool.tile([P, S * HEADS], f32, tag="epB")
                ep3 = ep[:].rearrange("p (s h) -> p s h", s=S)
                nc.vector.tensor_tensor(
                    out=ep3, in0=asrc_v,
                    in1=adst_ps[:].rearrange("p (s h) -> p s h", s=S),
                    op=OP.add)
                w1t = wpool.tile([P, S * HEADS], f32, tag="w1B")
                nc.scalar.activation(out=w1t[:], in_=ep[:], func=AF.Exp)
                w2t = wpool.tile([P, S * HEADS], f32, tag="w2B")
                nc.scalar.activation(out=w2t[:], in_=ep[:], func=AF.Exp, scale=NEG)
                # wb -> overwrite the asrc slot (cols 128:136 bf16) per tile
                nc.vector.tensor_tensor(
                    out=g3[:, :, 128:136], in0=w1t[:], in1=w2t[:], op=OP.max)
                # weight messages in place: H *= w (broadcast over 16 channels)
                nc.vector.tensor_tensor(
                    out=g3[:, :, 0:128].rearrange("p s (h c) -> p s h c", h=HEADS),
                    in0=g3[:, :, 0:128].rearrange("p s (h c) -> p s h c", h=HEADS),
                    in1=g3[:, :, 128:136][:, :, :, None].to_broadcast(
                        [P, S, HEADS, NHID]),
                    op=OP.mult)

                acc_ps = ps.tile([P, 136], f32, space="PSUM", tag="t144")
                for t in range(S):
                    nc.tensor.matmul(
                        acc_ps[:],
                        lhsT=ident_bf[:] if t == 0 else ind3[:, t - 1, :],
                        rhs=g3[:, t, 0:136],
                        start=(t == 0), stop=(t == S - 1))

                # epilogue: normalize, ELU proxy q, T2 row
                den = wpool.tile([P, HEADS], f32, tag="denB")
                nc.vector.tensor_scalar(out=den[:], in0=acc_ps[:, 128:136],
                                        scalar1=1e-30, scalar2=None, op0=OP.add)
                rec = wpool.tile([P, HEADS], f32, tag="recB")
                nc.vector.reciprocal(out=rec[:], in_=den[:])
                o1 = wpool.tile([P, NFEAT], f32, tag="o1B")
                nc.vector.tensor_tensor(
                    out=o1[:].rearrange("p (h c) -> p h c", h=HEADS),
                    in0=acc_ps[:, 0:128].rearrange("p (h c) -> p h c", h=HEADS),
                    in1=rec[:][:, :, None].to_broadcast([P, HEADS, NHID]),
                    op=OP.mult)
                mneg = wpool.tile([P, NFEAT], f32, tag="mnB")
                nc.vector.tensor_scalar(out=mneg[:], in0=o1[:], scalar1=0.0,
                                        scalar2=None, op0=OP.min)
                emn = wpool.tile([P, NFEAT], f32, tag="emB")
                nc.scalar.activation(out=emn[:], in_=mneg[:], func=AF.Exp)
                q = wpool.tile([P, NFEAT], f32, tag="qB")
                nc.vector.tensor_scalar(out=q[:], in0=o1[:], scalar1=0.0,
                                        scalar2=None, op0=OP.max)
                nc.vector.tensor_tensor(out=q[:], in0=q[:], in1=emn[:], op=OP.add)
                qt_ps = ps.tile([P, P], f32, space="PSUM", tag="t128")
                nc.tensor.transpose(out=qt_ps[:], in_=q[:], identity=ident[:])
                qt_sb = wpool.tile([P, P], f32, tag="qtsB")
                nc.vector.tensor_copy(out=qt_sb[:], in_=qt_ps[:])
                t2_ps = ps.tile([P, 4], f32, space="PSUM", tag="t4", bufs=1)
                nc.tensor.matmul(t2_ps[:], lhsT=qt_sb[:], rhs=w2e_sb[:],
                                 start=True, stop=False)
                nc.tensor.matmul(t2_ps[:], lhsT=ones1[:], rhs=b2_sb[:],
                                 start=False, stop=True)
                t2_sb = wpool.tile([P, 4], f32, tag="t2sB")
                nc.scalar.activation(out=t2_sb[:], in_=t2_ps[:], func=AF.Copy)
                nc.sync.dma_start(out=T2_loc[j * P:(j + 1) * P, :], in_=t2_sb[:])

            # ---- phase C: AllGather T2
            nc.gpsimd.collective_compute(
                "AllGather", mybir.AluOpType.bypass,
                ins=[T2_loc[:, :]], outs=[T2_full[:, :]],
                replica_groups=[list(range(NCORES))])

            # ---- phase D: layer-2 edge aggregation + log-softmax
            for j in range(CPC):
                S = tiles_per_slot[j] + 1     # +1: implicit self-loop tile
                tb = tile_base[j]
                a2d = wpool.tile([P, 1], f32, tag="a2dD")
                nc.sync.dma_start(out=a2d[:], in_=T2_loc[j * P:(j + 1) * P, 3:4])
                a2db = wpool.tile([P, 1], bf16, tag="a2dbD")
                nc.vector.tensor_copy(out=a2db[:], in_=a2d[:])

                g2 = gpool.tile([P, S * 4], f32, tag="g2D")
                g23 = g2[:].rearrange("p (s e) -> p s e", s=S)
                ind = gpool.tile([P, (S - 1) * P], bf16, tag="indB")
                ind3 = ind[:].rearrange("p (s n) -> p s n", s=S - 1)
                a2d_ps = ps.tile([P, S], f32, space="PSUM", tag="tadst")

                nc.sync.dma_start(out=g23[:, 0, :], in_=T2_loc[j * P:(j + 1) * P, :])
                nc.vector.tensor_copy(out=a2d_ps[:, 0:1], in_=a2d[:])
                for t in range(1, S):
                    nc.gpsimd.indirect_dma_start(
                        out=g23[:, t, :], out_offset=None, in_=T2_full[:, :],
                        in_offset=bass.IndirectOffsetOnAxis(
                            ap=offs_sb[:, tb + t - 1:tb + t], axis=0))
                    nc.vector.tensor_scalar(
                        out=ind3[:, t - 1, :], in0=iota_sb[:],
                        scalar1=dstl_sb[:, tb + t - 1:tb + t], scalar2=None,
                        op0=OP.is_equal)
                    indt_ps = ps.tile([P, P], bf16, space="PSUM", tag="t128")
                    nc.tensor.transpose(out=indt_ps[:], in_=ind3[:, t - 1, :],
                                        identity=ident_bf[:])
                    indt = wpool.tile([P, P], bf16, tag="indtD")
                    nc.vector.tensor_copy(out=indt[:], in_=indt_ps[:])
                    nc.tensor.matmul(a2d_ps[:, t:t + 1], lhsT=indt[:], rhs=a2db[:],
                                     start=True, stop=True)

                ep = wpool.tile([P, S], f32, tag="epD")
                nc.vector.tensor_tensor(
                    out=ep[:], in0=g23[:, :, 2:3].rearrange("p s e -> p (s e)"),
                    in1=a2d_ps[:], op=OP.add)
                w1t = wpool.tile([P, S], f32, tag="w1D")
                nc.scalar.activation(out=w1t[:], in_=ep[:], func=AF.Exp)
                w2t = wpool.tile([P, S], f32, tag="w2D")
                nc.scalar.activation(out=w2t[:], in_=ep[:], func=AF.Exp, scale=NEG)
                wmax = wpool.tile([P, S], f32, tag="wmD")
                nc.vector.tensor_tensor(out=wmax[:], in0=w1t[:], in1=w2t[:], op=OP.max)

                rhs2 = wpool.tile([P, S * 3], bf16, tag="rhs2D")
                r23 = rhs2[:].rearrange("p (s e) -> p s e", s=S)
                nc.vector.tensor_tensor(
                    out=r23[:, :, 0:2], in0=g23[:, :, 0:2],
                    in1=wmax[:][:, :, None].to_broadcast([P, S, 2]), op=OP.mult)
                nc.vector.tensor_copy(
                    out=r23[:, :, 2:3].rearrange("p s e -> p (s e)"), in_=wmax[:])

                acc2 = ps.tile([P, 3], f32, space="PSUM", tag="t144")
                for t in range(S):
                    nc.tensor.matmul(
                        acc2[:],
                        lhsT=ident_bf[:] if t == 0 else ind3[:, t - 1, :],
                        rhs=r23[:, t, :],
                        start=(t == 0), stop=(t == S - 1))

                den2 = wpool.tile([P, 1], f32, tag="den2D")
                nc.vector.tensor_scalar(out=den2[:], in0=acc2[:, 2:3],
                                        scalar1=1e-30, scalar2=None, op0=OP.add)
                rec2 = wpool.tile([P, 1], f32, tag="rec2D")
                nc.vector.reciprocal(out=rec2[:], in_=den2[:])
                o2 = wpool.tile([P, 2], f32, tag="o2D")
                nc.vector.tensor_tensor(
                    out=o2[:], in0=acc2[:, 0:2],
                    in1=rec2[:].to_broadcast([P, 2]), op=OP.mult)
                eo = wpool.tile([P, 2], f32, tag="eoD")
                nc.scalar.activation(out=eo[:], in_=o2[:], func=AF.Exp)
                ssum = wpool.tile([P, 1], f32, tag="ssD")
                nc.vector.reduce_sum(out=ssum[:], in_=eo[:], axis=mybir.AxisListType.X)
                lse = wpool.tile([P, 1], f32, tag="lseD")
                nc.scalar.activation(out=lse[:], in_=ssum[:], func=AF.Ln)
                fo = wpool.tile([P, 2], f32, tag="foD")
                nc.vector.tensor_scalar(out=fo[:], in0=o2[:], scalar1=lse[:, :1],
                                        scalar2=None, op0=OP.subtract)
                nc.sync.dma_start(out=out_ext[j * P:(j + 1) * P, :], in_=fo[:])

    _split_waits(nc, mybir)
    return nc


def _split_waits(nc, mybir):
    """Walrus here allows at most ONE sync-wait per instruction: move extra
    waits onto same-engine NoOps inserted right before the instruction."""
    ctr = [0]
    for func in nc.m.functions:
        for blk in func.blocks:
            out, changed = [], False
            for inst in blk.instructions:
                si = inst.sync_info
                waits = list(si.on_wait) if (si and si.on_wait) else []
                if len(waits) > 1:
                    changed = True
                    for w in waits[:-1]:
                        ctr[0] += 1
                        nop = mybir.InstNoOp(name=f"I-wsplit-{ctr[0]}", ins=[], outs=[])
                        nop.engine = inst.engine
                        nop.sync_info = mybir.SyncInfo(on_wait=[w], on_update=[])
                        out.append(nop)
                    si.on_wait = [waits[-1]]
                out.append(inst)
            if changed:
                blk.instructions = out


# ------------------------------------------------------------------- entry
def kernel(**inputs):
    import sys
    if "/opt/trn_rl_repo" not in sys.path:
        sys.path.insert(0, "/opt/trn_rl_repo")
    from concourse.bass_utils import run_bass_kernel_spmd

    x = np.asarray(inputs["x"], np.float32)
    ei = np.asarray(inputs["edge_index"]).astype(np.int64)
    W1 = np.asarray(inputs["W1"], np.float32)
    as1 = np.asarray(inputs["att_src1"], np.float32)
    ad1 = np.asarray(inputs["att_dst1"], np.float32)
    b1 = np.asarray(inputs["bias1"], np.float32)
    W2 = np.asarray(inputs["W2"], np.float32)
    as2 = np.asarray(inputs["att_src2"], np.float32)
    ad2 = np.asarray(inputs["att_dst2"], np.float32)
    b2 = np.asarray(inputs["bias2"], np.float32)

    meta, arr = _host_prep(x, ei, W1, as1, ad1, b1, W2, as2, ad2, b2)

    key = (meta["Ttot"], meta["tiles_per_slot"])
    if key not in _PROG_CACHE:
        _PROG_CACHE[key] = _build_program(meta)
    nc = _PROG_CACHE[key]

    core_ids = list(range(NCORES))
    in_maps = []
    for c in core_ids:
        in_maps.append(dict(
            x_loc=arr["x_loc"][c], W1_ext=arr["W1_ext"], b1row=arr["b1row"],
            W2_ext=arr["W2_ext"], b2row=arr["b2row"], iota_row=arr["iota_row"],
            offs_src=arr["offs_src"][c], dstl=arr["dstl"][c],
        ))
    res = run_bass_kernel_spmd(nc, in_maps, core_ids)
    out = np.concatenate([res.results[c]["out"] for c in core_ids], axis=0)
    return np.ascontiguousarray(out[:N])


# revision 11
# speedup vs baseline: 1.0474x; 1.0474x over previous
"""Bass/Trainium2 kernel for 2-layer GAT (nn_GAT_18640158065247).

Strategy (8 NeuronCores, SPMD single program):
 - Host: add self-loops, sort edges by dst, shard dst-contiguously so each core
   owns 49 aligned 128-node chunks (50176 padded nodes) and the edges into them.
 - Phase A (replicated on every core): T[n] = [H bf16 | alpha_src f32] table,
   AD[n] = alpha_dst bf16 table, via X^T PE-transpose + fused matmul with
   W1_ext = [W1 | W1@a_src | W1@a_dst] and a rank-1 bias row.
 - Phase B (edge-parallel): per 128-node chunk, per 128-edge tile:
   indirect-gather T rows by src, build one-hot IND (dst-local vs iota),
   expand alpha_dst to edges via PE matmul with IND^T (DVE stream transpose),
   w = max(exp(e), exp(0.2 e)) [= exp(leaky_relu(e))], weight messages, and
   accumulate [Σ w·H | Σ w] into PSUM with one matmul per tile.
   Epilogue: normalize, ELU (as q = exp(min(x,0)) + max(x,0), with the -1
   folded into the layer-2 bias row), z2/alpha2 via a second fused matmul.
 - AllGather the tiny per-node layer-2 table T2 = [z2+b2 | a2src | a2dst].
 - Phase D: same edge loop for layer 2 on 16-byte rows, then log-softmax.
"""
import numpy as np

N = 50000
E = 800000
NFEAT = 128
NHID = 16
HEADS = 8
NCLASS = 2
NEG = 0.2
NCORES = 8
P = 128

_PROG_CACHE = {}


# ---------------------------------------------------------------- host helpers
def _host_prep(x, edge_index, W1, att_src1, att_dst1, bias1, W2, att_src2,
               att_dst2, bias2):
    n_chunks = -(-N // P)                      # 391 -> pad to multiple of NCORES
    n_chunks = -(-n_chunks // NCORES) * NCORES  # 392
    NP = n_chunks * P                           # 50176
    CPC = n_chunks // NCORES                    # 49
    NPC = CPC * P                               # 6272 nodes per core

    # the appended self-loops (one per node) are handled as an implicit,
    # gather-free tile per chunk; only the raw edges go through the sort.
    src = edge_index[0]
    dst = edge_index[1]
    order = np.argsort(dst, kind="stable")
    src_s = src[order].astype(np.int32)
    dst_s = dst[order].astype(np.int32)

    chunk_of_edge = dst_s // P                          # [Etot]
    counts = np.bincount(chunk_of_edge, minlength=n_chunks)
    starts = np.zeros(n_chunks + 1, np.int64)
    np.cumsum(counts, out=starts[1:])

    cnt_cj = counts.reshape(NCORES, CPC)
    S_slots = (-(-cnt_cj.max(axis=0) // P) * P).astype(np.int64)   # padded edges/slot
    tiles_per_slot = S_slots // P
    tile_base = np.zeros(CPC + 1, np.int64)
    np.cumsum(tiles_per_slot, out=tile_base[1:])
    Ttot = int(tile_base[-1])                     # tiles per core

    offs_src = np.zeros((NCORES, P, Ttot), np.int32)
    dstl = np.full((NCORES, P, Ttot), -1.0, np.float32)
    for c in range(NCORES):
        for j in range(CPC):
            g = c * CPC + j
            e0, cnt = starts[g], counts[g]
            Sp = int(S_slots[j])
            sl = np.zeros(Sp, np.int32)
            dl = np.full(Sp, -1.0, np.float32)
            sl[:cnt] = src_s[e0:e0 + cnt]
            dl[:cnt] = (dst_s[e0:e0 + cnt] - g * P).astype(np.float32)
            tb = int(tile_base[j])
            nt = Sp // P
            offs_src[c, :, tb:tb + nt] = sl.reshape(nt, P).T
            dstl[c, :, tb:tb + nt] = dl.reshape(nt, P).T

    # parameters (fused)
    va_src1 = (W1.reshape(NFEAT, HEADS, NHID) * att_src1[None]).sum(-1)  # [128,8]
    va_dst1 = (W1.reshape(NFEAT, HEADS, NHID) * att_dst1[None]).sum(-1)
    W1_ext = np.concatenate([W1, va_src1, va_dst1], axis=1).astype(np.float32)  # [128,144]
    b1row = np.concatenate([bias1, np.zeros(16, np.float32)])[None, :].astype(np.float32)

    va2s = W2 @ att_src2[0]            # [128]
    va2d = W2 @ att_dst2[0]
    W2_ext = np.concatenate([W2, va2s[:, None], va2d[:, None]], axis=1).astype(np.float32)  # [128,4]
    bias_ext2 = np.concatenate([bias2, np.zeros(2, np.float32)])
    b2row = (bias_ext2 - W2_ext.sum(0))[None, :].astype(np.float32)   # folds q-1 shift

    import ml_dtypes
    x_pad = np.zeros((NP, NFEAT), np.float32)
    x_pad[:N] = x
    x_loc = x_pad.reshape(NCORES, NPC, NFEAT).astype(ml_dtypes.bfloat16)
    W1_ext_bf = W1_ext.astype(ml_dtypes.bfloat16)
    b1row_bf = b1row.astype(ml_dtypes.bfloat16)
    iota_row = np.tile(np.arange(P, dtype=np.float32), (P, 1))

    meta = dict(NP=NP, CPC=CPC, NPC=NPC, Ttot=Ttot,
                tiles_per_slot=tuple(int(t) for t in tiles_per_slot),
                tile_base=tuple(int(t) for t in tile_base))
    arrays = dict(offs_src=offs_src, dstl=dstl,
                  W1_ext=W1_ext_bf, b1row=b1row_bf, W2_ext=W2_ext, b2row=b2row,
                  x_loc=x_loc, iota_row=iota_row)
    return meta, arrays


# ---------------------------------------------------------------- device build
def _build_program(meta):
    import sys
    if "/opt/trn_rl_repo" not in sys.path:
        sys.path.insert(0, "/opt/trn_rl_repo")
    import concourse.bass as bass
    import concourse.tile as tile
    from concourse import mybir
    from concourse.masks import make_identity

    f32 = mybir.dt.float32
    bf16 = mybir.dt.bfloat16
    i32 = mybir.dt.int32
    AF = mybir.ActivationFunctionType
    OP = mybir.AluOpType

    NP = meta["NP"]; CPC = meta["CPC"]; NPC = meta["NPC"]; Ttot = meta["Ttot"]
    tiles_per_slot = meta["tiles_per_slot"]; tile_base = meta["tile_base"]
    n_chunks = NP // P

    nc = bass.Bass(num_devices=NCORES)

    x_in = nc.dram_tensor("x_loc", [NPC, NFEAT], bf16, kind="ExternalInput")
    w1e_in = nc.dram_tensor("W1_ext", [NFEAT, 144], bf16, kind="ExternalInput")
    b1_in = nc.dram_tensor("b1row", [1, 144], bf16, kind="ExternalInput")
    w2e_in = nc.dram_tensor("W2_ext", [NFEAT, 4], f32, kind="ExternalInput")
    b2_in = nc.dram_tensor("b2row", [1, 4], f32, kind="ExternalInput")
    iota_in = nc.dram_tensor("iota_row", [P, P], f32, kind="ExternalInput")
    offs_in = nc.dram_tensor("offs_src", [P, Ttot], i32, kind="ExternalInput")
    dstl_in = nc.dram_tensor("dstl", [P, Ttot], f32, kind="ExternalInput")

    out_ext = nc.dram_tensor("out", [NPC, NCLASS], f32, kind="ExternalOutput")

    # internal DRAM
    T_loc = nc.dram_tensor("T_loc", [NPC, 144], bf16)
    T_tab = nc.dram_tensor("T_tab", [NP, 144], bf16, addr_space="Shared")
    AD_loc = nc.dram_tensor("AD_loc", [NPC, HEADS], bf16)
    T2_loc = nc.dram_tensor("T2_loc", [NPC, 4], f32)
    T2_full = nc.dram_tensor("T2_full", [NP, 4], f32, addr_space="Shared")

    with tile.TileContext(nc) as tc:
        with (
            tc.tile_pool(name="const", bufs=1) as cpool,
            tc.tile_pool(name="work", bufs=3) as wpool,
            tc.tile_pool(name="gath", bufs=2) as gpool,
            tc.tile_pool(name="ps", bufs=2, space="PSUM") as ps,
        ):
            # ---- constants
            w1e_sb = cpool.tile([NFEAT, 144], bf16)
            nc.sync.dma_start(out=w1e_sb[:], in_=w1e_in[:, :])
            b1_sb = cpool.tile([1, 144], bf16)
            nc.sync.dma_start(out=b1_sb[:], in_=b1_in[:, :])
            w2e_sb = cpool.tile([NFEAT, 4], f32)
            nc.sync.dma_start(out=w2e_sb[:], in_=w2e_in[:, :])
            b2_sb = cpool.tile([1, 4], f32)
            nc.sync.dma_start(out=b2_sb[:], in_=b2_in[:, :])
            iota_sb = cpool.tile([P, P], f32)
            nc.sync.dma_start(out=iota_sb[:], in_=iota_in[:, :])
            offs_sb = cpool.tile([P, Ttot], i32)
            nc.sync.dma_start(out=offs_sb[:], in_=offs_in[:, :])
            dstl_sb = cpool.tile([P, Ttot], f32)
            nc.sync.dma_start(out=dstl_sb[:], in_=dstl_in[:, :])
            ident = cpool.tile([P, P], f32)
            make_identity(nc, ident[:])
            ident_bf = cpool.tile([P, P], bf16)
            make_identity(nc, ident_bf[:])
            ones1 = cpool.tile([1, P], f32)
            nc.vector.memset(ones1[:], 1.0)
            ones1_bf = cpool.tile([1, P], bf16)
            nc.vector.memset(ones1_bf[:], 1.0)

            # ---- phase A: build T / AD tables (own shard only, then AllGather)
            # grouped by GA chunks per iteration to amortize HWDGE DMA overhead
            GA = 7
            for j0 in range(0, CPC, GA):
                ga = min(GA, CPC - j0)
                nb = j0 * P
                xt_sb = wpool.tile([P, ga * P], bf16, tag="xtA")
                nc.sync.dma_start(out=xt_sb[:], in_=x_in[nb:nb + ga * P, :],
                                  transpose=True)
                t_sb = wpool.tile([P, ga * 144], bf16, tag="tsbA")
                ad_sb = wpool.tile([P, ga * HEADS], bf16, tag="adA")
                for k in range(ga):
                    t_ps = ps.tile([P, 144], f32, space="PSUM", tag="t144")
                    nc.tensor.matmul(t_ps[:], lhsT=xt_sb[:, k * P:(k + 1) * P],
                                     rhs=w1e_sb[:], start=True, stop=False)
                    nc.tensor.matmul(t_ps[:], lhsT=ones1_bf[:], rhs=b1_sb[:],
                                     start=False, stop=True)
                    nc.vector.tensor_copy(
                        out=t_sb[:, k * 144:k * 144 + 128], in_=t_ps[:, 0:128])
                    nc.vector.tensor_copy(
                        out=t_sb[:, k * 144 + 128:(k + 1) * 144].bitcast(f32),
                        in_=t_ps[:, 128:136])
                    nc.scalar.activation(
                        out=ad_sb[:, k * HEADS:(k + 1) * HEADS],
                        in_=t_ps[:, 136:144], func=AF.Copy)
                nc.sync.dma_start(
                    out=T_loc[nb:nb + ga * P, :].rearrange("(c p) f -> p c f", p=P),
                    in_=t_sb[:].rearrange("p (c f) -> p c f", c=ga))
                nc.sync.dma_start(
                    out=AD_loc[nb:nb + ga * P, :].rearrange("(c p) f -> p c f", p=P),
                    in_=ad_sb[:].rearrange("p (c f) -> p c f", c=ga))

            nc.gpsimd.collective_compute(
                "AllGather", mybir.AluOpType.bypass,
                ins=[T_loc[:, :]], outs=[T_tab[:, :]],
                replica_groups=[list(range(NCORES))])

            # ---- phase B: layer-1 edge aggregation over own 49 chunks
            for j in range(CPC):
                S = tiles_per_slot[j] + 1     # +1: implicit self-loop tile
                tb = tile_base[j]
                adc = wpool.tile([P, HEADS], bf16, tag="adcB")
                nc.sync.dma_start(out=adc[:], in_=AD_loc[j * P:(j + 1) * P, :])

                gbuf = gpool.tile([P, S * 144], bf16, tag="gB")
                g3 = gbuf[:].rearrange("p (s e) -> p s e", s=S)
                ind = gpool.tile([P, (S - 1) * P], bf16, tag="indB")
                ind3 = ind[:].rearrange("p (s n) -> p s n", s=S - 1)
                adst_ps = ps.tile([P, S * HEADS], f32, space="PSUM", tag="tadst")

                # tile 0: this chunk's self-loops — data is local, one-hot is I
                nc.sync.dma_start(out=g3[:, 0, :], in_=T_loc[j * P:(j + 1) * P, :])
                nc.vector.tensor_copy(out=adst_ps[:, 0:HEADS], in_=adc[:])
                for t in range(1, S):
                    nc.gpsimd.indirect_dma_start(
                        out=g3[:, t, :], out_offset=None, in_=T_tab[:, :],
                        in_offset=bass.IndirectOffsetOnAxis(
                            ap=offs_sb[:, tb + t - 1:tb + t], axis=0))
                    nc.vector.tensor_scalar(
                        out=ind3[:, t - 1, :], in0=iota_sb[:],
                        scalar1=dstl_sb[:, tb + t - 1:tb + t], scalar2=None,
                        op0=OP.is_equal)
                    indt_ps = ps.tile([P, P], bf16, space="PSUM", tag="t128")
                    nc.tensor.transpose(out=indt_ps[:], in_=ind3[:, t - 1, :],
                                        identity=ident_bf[:])
                    indt = wpool.tile([P, P], bf16, tag="indtB")
                    nc.vector.tensor_copy(out=indt[:], in_=indt_ps[:])
                    nc.tensor.matmul(adst_ps[:, t * HEADS:(t + 1) * HEADS],
                                     lhsT=indt[:], rhs=adc[:],
                                     start=True, stop=True)

                # e = asrc + adst (all tiles at once), w = max(exp(e), exp(.2e))
                asrc_v = g3[:, :, 128:144].bitcast(f32)       # [P, S, 8]
                ep = wpool.tile([P, S * HEADS], f32, tag="epB")
                ep3 = ep[:].rearrange("p (s h) -> p s h", s=S)
                nc.vector.tensor_tensor(
                    out=ep3, in0=asrc_v,
                    in1=adst_ps[:].rearrange("p (s h) -> p s h", s=S),
                    op=OP.add)
                w1t = wpool.tile([P, S * HEADS], f32, tag="w1B")
                nc.scalar.activation(out=w1t[:], in_=ep[:], func=AF.Exp)
                w2t = wpool.tile([P, S * HEADS], f32, tag="w2B")
                nc.scalar.activation(out=w2t[:], in_=ep[:], func=AF.Exp, scale=NEG)
                # wb -> overwrite the asrc slot (cols 128:136 bf16) per tile
                nc.vector.tensor_tensor(
                    out=g3[:, :, 128:136], in0=w1t[:], in1=w2t[:], op=OP.max)
                # weight messages in place: H *= w (broadcast over 16 channels)
                nc.vector.tensor_tensor(
                    out=g3[:, :, 0:128].rearrange("p s (h c) -> p s h c", h=HEADS),
                    in0=g3[:, :, 0:128].rearrange("p s (h c) -> p s h c", h=HEADS),
                    in1=g3[:, :, 128:136][:, :, :, None].to_broadcast(
                        [P, S, HEADS, NHID]),
                    op=OP.mult)

                acc_ps = ps.tile([P, 136], f32, space="PSUM", tag="t144")
                for t in range(S):
                    nc.tensor.matmul(
                        acc_ps[:],
                        lhsT=ident_bf[:] if t == 0 else ind3[:, t - 1, :],
                        rhs=g3[:, t, 0:136],
                        start=(t == 0), stop=(t == S - 1))

                # epilogue: normalize, ELU proxy q, T2 row
                den = wpool.tile([P, HEADS], f32, tag="denB")
                nc.vector.tensor_scalar(out=den[:], in0=acc_ps[:, 128:136],
                                        scalar1=1e-30, scalar2=None, op0=OP.add)
                rec = wpool.tile([P, HEADS], f32, tag="recB")
                nc.vector.reciprocal(out=rec[:], in_=den[:])
                o1 = wpool.tile([P, NFEAT], f32, tag="o1B")
                nc.vector.tensor_tensor(
                    out=o1[:].rearrange("p (h c) -> p h c", h=HEADS),
                    in0=acc_ps[:, 0:128].rearrange("p (h c) -> p h c", h=HEADS),
                    in1=rec[:][:, :, None].to_broadcast([P, HEADS, NHID]),
                    op=OP.mult)
                mneg = wpool.tile([P, NFEAT], f32, tag="mnB")
                nc.vector.tensor_scalar(out=mneg[:], in0=o1[:], scalar1=0.0,
                                        scalar2=None, op0=OP.min)
                emn = wpool.tile([P, NFEAT], f32, tag="emB")
                nc.scalar.activation(out=emn[:], in_=mneg[:], func=AF.Exp)
                q = wpool.tile([P, NFEAT], f32, tag="qB")
                nc.vector.tensor_scalar(out=q[:], in0=o1[:], scalar1=0.0,
                                        scalar2=None, op0=OP.max)
                nc.vector.tensor_tensor(out=q[:], in0=q[:], in1=emn[:], op=OP.add)
                qt_ps = ps.tile([P, P], f32, space="PSUM", tag="t128")
                nc.tensor.transpose(out=qt_ps[:], in_=q[:], identity=ident[:])
                qt_sb = wpool.tile([P, P], f32, tag="qtsB")
                nc.vector.tensor_copy(out=qt_sb[:], in_=qt_ps[:])
                t2_ps = ps.tile([P, 4], f32, space="PSUM", tag="t4", bufs=1)
                nc.tensor.matmul(t2_ps[:], lhsT=qt_sb[:], rhs=w2e_sb[:],
                                 start=True, stop=False)
                nc.tensor.matmul(t2_ps[:], lhsT=ones1[:], rhs=b2_sb[:],
                                 start=False, stop=True)
                t2_sb = wpool.tile([P, 4], f32, tag="t2sB")
                nc.scalar.activation(out=t2_sb[:], in_=t2_ps[:], func=AF.Copy)
                nc.sync.dma_start(out=T2_loc[j * P:(j + 1) * P, :], in_=t2_sb[:])

            # ---- phase C: AllGather T2
            nc.gpsimd.collective_compute(
                "AllGather", mybir.AluOpType.bypass,
                ins=[T2_loc[:, :]], outs=[T2_full[:, :]],
                replica_groups=[list(range(NCORES))])

            # ---- phase D: layer-2 edge aggregation + log-softmax
            for j in range(CPC):
                S = tiles_per_slot[j] + 1     # +1: implicit self-loop tile
                tb = tile_base[j]
                a2d = wpool.tile([P, 1], f32, tag="a2dD")
                nc.sync.dma_start(out=a2d[:], in_=T2_loc[j * P:(j + 1) * P, 3:4])
                a2db = wpool.tile([P, 1], bf16, tag="a2dbD")
                nc.vector.tensor_copy(out=a2db[:], in_=a2d[:])

                g2 = gpool.tile([P, S * 4], f32, tag="g2D")
                g23 = g2[:].rearrange("p (s e) -> p s e", s=S)
                ind = gpool.tile([P, (S - 1) * P], bf16, tag="indB")
                ind3 = ind[:].rearrange("p (s n) -> p s n", s=S - 1)
                a2d_ps = ps.tile([P, S], f32, space="PSUM", tag="tadst")

                nc.sync.dma_start(out=g23[:, 0, :], in_=T2_loc[j * P:(j + 1) * P, :])
                nc.vector.tensor_copy(out=a2d_ps[:, 0:1], in_=a2d[:])
                for t in range(1, S):
                    nc.gpsimd.indirect_dma_start(
                        out=g23[:, t, :], out_offset=None, in_=T2_full[:, :],
                        in_offset=bass.IndirectOffsetOnAxis(
                            ap=offs_sb[:, tb + t - 1:tb + t], axis=0))
                    nc.vector.tensor_scalar(
                        out=ind3[:, t - 1, :], in0=iota_sb[:],
                        scalar1=dstl_sb[:, tb + t - 1:tb + t], scalar2=None,
                        op0=OP.is_equal)
                    indt_ps = ps.tile([P, P], bf16, space="PSUM", tag="t128")
                    nc.tensor.transpose(out=indt_ps[:], in_=ind3[:, t - 1, :],
                                        identity=ident_bf[:])
                    indt = wpool.tile([P, P], bf16, tag="indtD")
                    nc.vector.tensor_copy(out=indt[:], in_=indt_ps[:])
                    nc.tensor.matmul(a2d_ps[:, t:t + 1], lhsT=indt[:], rhs=a2db[:],
                                     start=True, stop=True)

                ep = wpool.tile([P, S], f32, tag="epD")
                nc.vector.tensor_tensor(
                    out=ep[:], in0=g23[:, :, 2:3].rearrange("p s e -> p (s e)"),
                    in1=a2d_ps[:], op=OP.add)
                w1t = wpool.tile([P, S], f32, tag="w1D")
                nc.scalar.activation(out=w1t[:], in_=ep[:], func=AF.Exp)
                w2t = wpool.tile([P, S], f32, tag="w2D")
                nc.scalar.activation(out=w2t[:], in_=ep[:], func=AF.Exp, scale=NEG)
                wmax = wpool.tile([P, S], f32, tag="wmD")
                nc.vector.tensor_tensor(out=wmax[:], in0=w1t[:], in1=w2t[:], op=OP.max)

                rhs2 = wpool.tile([P, S * 3], bf16, tag="rhs2D")
                r23 = rhs2[:].rearrange("p (s e) -> p s e", s=S)
                nc.vector.tensor_tensor(
                    out=r23[:, :, 0:2], in0=g23[:, :, 0:2],
                    in1=wmax[:][:, :, None].to_broadcast([P, S, 2]), op=OP.mult)
                nc.vector.tensor_copy(
                    out=r23[:, :, 2:3].rearrange("p s e -> p (s e)"), in_=wmax[:])

                acc2 = ps.tile([P, 3], f32, space="PSUM", tag="t144")
                for t in range(S):
                    nc.tensor.matmul(
                        acc2[:],
                        lhsT=ident_bf[:] if t == 0 else ind3[:, t - 1, :],
                        rhs=r23[:, t, :],
                        start=(t == 0), stop=(t == S - 1))

                den2 = wpool.tile([P, 1], f32, tag="den2D")
                nc.vector.tensor_scalar(out=den2[:], in0=acc2[:, 2:3],
                                        scalar1=1e-30, scalar2=None, op0=OP.add)
                rec2 = wpool.tile([P, 1], f32, tag="rec2D")
                nc.vector.reciprocal(out=rec2[:], in_=den2[:])
                o2 = wpool.tile([P, 2], f32, tag="o2D")
                nc.vector.tensor_tensor(
                    out=o2[:], in0=acc2[:, 0:2],
                    in1=rec2[:].to_broadcast([P, 2]), op=OP.mult)
                eo = wpool.tile([P, 2], f32, tag="eoD")
                nc.scalar.activation(out=eo[:], in_=o2[:], func=AF.Exp)
                ssum = wpool.tile([P, 1], f32, tag="ssD")
                nc.vector.reduce_sum(out=ssum[:], in_=eo[:], axis=mybir.AxisListType.X)
                lse = wpool.tile([P, 1], f32, tag="lseD")
                nc.scalar.activation(out=lse[:], in_=ssum[:], func=AF.Ln)
                fo = wpool.tile([P, 2], f32, tag="foD")
                nc.vector.tensor_scalar(out=fo[:], in0=o2[:], scalar1=lse[:, :1],
                                        scalar2=None, op0=OP.subtract)
                nc.sync.dma_start(out=out_ext[j * P:(j + 1) * P, :], in_=fo[:])

    _split_waits(nc, mybir)
    return nc


def _split_waits(nc, mybir):
    """Walrus here allows at most ONE sync-wait per instruction: move extra
    waits onto same-engine NoOps inserted right before the instruction."""
    ctr = [0]
    for func in nc.m.functions:
        for blk in func.blocks:
            out, changed = [], False
            for inst in blk.instructions:
                si = inst.sync_info
                waits = list(si.on_wait) if (si and si.on_wait) else []
                if len(waits) > 1:
                    changed = True
                    for w in waits[:-1]:
                        ctr[0] += 1
                        nop = mybir.InstNoOp(name=f"I-wsplit-{ctr[0]}", ins=[], outs=[])
                        nop.engine = inst.engine
                        nop.sync_info = mybir.SyncInfo(on_wait=[w], on_update=[])
                        out.append(nop)
                    si.on_wait = [waits[-1]]
                out.append(inst)
            if changed:
                blk.instructions = out


# ------------------------------------------------------------------- entry
def kernel(**inputs):
    import sys
    if "/opt/trn_rl_repo" not in sys.path:
        sys.path.insert(0, "/opt/trn_rl_repo")
    from concourse.bass_utils import run_bass_kernel_spmd

    x = np.asarray(inputs["x"], np.float32)
    ei = np.asarray(inputs["edge_index"]).astype(np.int64)
    W1 = np.asarray(inputs["W1"], np.float32)
    as1 = np.asarray(inputs["att_src1"], np.float32)
    ad1 = np.asarray(inputs["att_dst1"], np.float32)
    b1 = np.asarray(inputs["bias1"], np.float32)
    W2 = np.asarray(inputs["W2"], np.float32)
    as2 = np.asarray(inputs["att_src2"], np.float32)
    ad2 = np.asarray(inputs["att_dst2"], np.float32)
    b2 = np.asarray(inputs["bias2"], np.float32)

    meta, arr = _host_prep(x, ei, W1, as1, ad1, b1, W2, as2, ad2, b2)

    key = (meta["Ttot"], meta["tiles_per_slot"])
    if key not in _PROG_CACHE:
        _PROG_CACHE[key] = _build_program(meta)
    nc = _PROG_CACHE[key]

    core_ids = list(range(NCORES))
    in_maps = []
    for c in core_ids:
        in_maps.append(dict(
            x_loc=arr["x_loc"][c], W1_ext=arr["W1_ext"], b1row=arr["b1row"],
            W2_ext=arr["W2_ext"], b2row=arr["b2row"], iota_row=arr["iota_row"],
            offs_src=arr["offs_src"][c], dstl=arr["dstl"][c],
        ))
    res = run_bass_kernel_spmd(nc, in_maps, core_ids)
    out = np.concatenate([res.results[c]["out"] for c in core_ids], axis=0)
    return np.ascontiguousarray(out[:N])
